# revision 16
# baseline (speedup 1.0000x reference)
"""Trainium2 Bass kernel for nn_ButterflyFactorNewMlp.

Computes: attn = einsum('ds,td->st', w1, w2) * sparse_mask
          out  = gelu(einsum('bds,st->bdt', x, attn) + b2)   (exact erf gelu)

Key structural facts (hardcoded):
  - x: [64, 768, 729] f32; w1: [2916, 729]; w2: [729, 2916]; b2: [729]
  - sparse_mask is block-diagonal: mask[s,t] != 0  iff  s//81 == t//81 and
    (s%27)//3 == (t%27)//3.  So attn = blockdiag of 9 diagonal 81x81 blocks.
  - Sharding: data-parallel on batch (8 batches/core); the attn computation
    is sharded over the hidden (d) dim with an on-device AllReduce of the
    9 diagonal blocks (236KB).

Contraction layout (s' space, 732 wide): the host inserts a ones column at
position 405 and appends another at 730, so after the on-chip transpose the
bias rows land at s'=405 (serving output cols 0:384) and s'=730 (cols
384:729).  Features s >= 405 shift by one.  With attn block-diagonal, the
output column range [0,384) only needs contraction rows s' < 406 and
[384,729) only rows s' >= 320, so each output half needs just 4 matmuls:

  half A (t 0:384):   s'-chunks 0,1,2 full (K=128) + chunk3 rows 0:22
  half B (t 383:729): chunk2 rows 64:128 + chunks 3,4 full + chunk5 rows 0:91

Per 128-token tile: 6 PE transposes (fp32r) put s' on partitions, 8 fp32r
matmuls contract against the block-diagonal attn (bias rows ride along via
the ones columns), exact-gelu via ScalarE LUT straight out of PSUM.
"""

import sys

if "/opt/trn_rl_repo" not in sys.path:
    sys.path.insert(0, "/opt/trn_rl_repo")

import numpy as np

import concourse.bacc as bacc
import concourse.bass as bass
import concourse.mybir as mybir
import concourse.tile as tile
from concourse.bass import ds, ts
from concourse.bass_utils import run_bass_kernel_spmd
from concourse.masks import make_identity

F32 = mybir.dt.float32
F32R = mybir.dt.float32r
GELU = mybir.ActivationFunctionType.Gelu

N_CORES = 8
B, D, S = 64, 768, 729          # batch, channels, features (729 = in = out)
H = 2916                        # hidden dim of the weight contraction
SP = 732                        # s' width: 729 + 2 ones columns + 1 pad
M_PER_CORE = (B // N_CORES) * D  # 6144 tokens per core
M_TILE = 128
M_BATCH = 2                      # m-tiles DMA'd per transfer (~750KB)
N_ITERS = M_PER_CORE // (M_TILE * M_BATCH)
HP = 3072                        # hidden padded to 8*384
HC = HP // N_CORES               # 384 hidden rows per core
N_KD = HC // 128                 # 3 contraction chunks for the attn matmuls
NBLK = 9                         # 81x81 diagonal blocks
BLK = 81
T_SIZES = [128, 128, 128, 128, 128, 92]  # transpose widths per s'-chunk
# main-matmul plan: (chunk j, partition base, K) per output half
MM_A = [(0, 0, 128), (1, 0, 128), (2, 0, 128), (3, 0, 22)]
MM_B = [(2, 64, 64), (3, 0, 128), (4, 0, 128), (5, 0, 91)]
# t split: both halves >= 256 keeps fp32r at full rate; fp32r needs an even
# psum free count and 729 is odd, so half B is 346 wide starting at 383
# (column 383 computed twice, copied out once)
T0 = 384
T1_OFF, T1 = 383, 346

_COMPILED = None
LAST = None  # BassKernelResults of the most recent kernel() call (for test.py)


def _build():
    nc = bacc.Bacc("TRN2", target_bir_lowering=False, debug=False)

    x_d = nc.dram_tensor("x", [M_PER_CORE, SP], F32, kind="ExternalInput")
    w1_d = nc.dram_tensor("w1c", [HC, S], F32, kind="ExternalInput")
    w2t_d = nc.dram_tensor("w2tc", [HC, S], F32, kind="ExternalInput")
    mb_d = nc.dram_tensor("maskb", [NBLK, BLK, BLK], F32, kind="ExternalInput")
    b2_d = nc.dram_tensor("b2", [1, S], F32, kind="ExternalInput")
    out_d = nc.dram_tensor("out", [M_PER_CORE, S], F32, kind="ExternalOutput")
    part_d = nc.dram_tensor("attn_part", [NBLK, BLK, BLK], F32)
    ar_d = nc.dram_tensor("attn_ar", [NBLK, BLK, BLK], F32, addr_space="Shared")

    with tile.TileContext(nc) as tc:
        with (
            tc.tile_pool(name="const", bufs=1) as cpool,
            tc.tile_pool(name="xin", bufs=4) as xpool,
            tc.tile_pool(name="xt", bufs=22) as xtpool,
            tc.tile_pool(name="oout", bufs=4) as opool,
            tc.tile_pool(name="tpsum", bufs=4, space="PSUM") as tpsum,
            tc.tile_pool(name="opsum", bufs=2, space="PSUM") as opsum,
        ):
            # ---------------- stage 1: attn blocks ----------------
            # fp32 matmuls here (fp32r forbids odd 81-wide outputs, and at
            # ~4us total the 4-cycle fp32 rate is irrelevant)
            w1_sb = cpool.tile([128, N_KD, S], F32)
            w2_sb = cpool.tile([128, N_KD, S], F32)
            # one DMA per 128-row chunk so the first attn matmuls can start
            # as soon as chunk 0 of both weights has landed
            for kd in range(N_KD):
                nc.sync.dma_start(w1_sb[:, kd, :], w1_d[ts(kd, 128), :])
                nc.sync.dma_start(w2_sb[:, kd, :], w2t_d[ts(kd, 128), :])
            mb_sb = cpool.tile([BLK, NBLK, BLK], F32)
            nc.sync.dma_start(mb_sb[:], mb_d[:].rearrange("b p f -> p b f"))

            part_sb = cpool.tile([BLK, NBLK, BLK], F32)
            for i in range(NBLK):
                ps_t = opsum.tile([BLK, BLK], F32, tag="psA")
                for kd in range(N_KD):
                    nc.tensor.matmul(
                        ps_t[:],
                        w1_sb[:, kd, ts(i, BLK)],
                        w2_sb[:, kd, ts(i, BLK)],
                        start=(kd == 0),
                        stop=(kd == N_KD - 1),
                    )
                nc.vector.tensor_tensor(
                    part_sb[:, i, :], ps_t[:], mb_sb[:, i, :], mybir.AluOpType.mult
                )
            nc.sync.dma_start(part_d[:].rearrange("b p f -> p b f"), part_sb[:])

            nc.gpsimd.collective_compute(
                "AllReduce",
                mybir.AluOpType.add,
                replica_groups=[list(range(N_CORES))],
                ins=[part_d[:]],
                outs=[ar_d[:]],
            )

            # dense block-diagonal attn in SBUF (f32r), in s' coordinates:
            # blocks 5-8 shift down one row; bias rows at s'=405 and s'=730
            attn_all = cpool.tile([128, 6, S], F32R)
            nc.gpsimd.memset(attn_all[:].bitcast(F32), 0.0)
            for i in range(NBLK):
                r0 = i * BLK + (1 if i >= 5 else 0)
                start = r0
                while start < r0 + BLK:
                    j = start // 128
                    end = min(r0 + BLK, (j + 1) * 128)
                    # cast f32 -> f32r during DMA (SWDGE); the rounding
                    # happens in the PE, identical to a DVE cast
                    nc.gpsimd.dma_start(
                        attn_all[start - j * 128 : end - j * 128, j, ts(i, BLK)],
                        ar_d[i, start - r0 : end - r0, :],
                    )
                    start = end
            nc.gpsimd.dma_start(attn_all[21:22, 3, 0:T0], b2_d[:, 0:T0])
            nc.gpsimd.dma_start(attn_all[90:91, 5, T0:S], b2_d[:, T0:S])

            # ---------------- stage 2: main matmul ----------------
            ident = cpool.tile([128, 128], F32)
            make_identity(nc, ident[:])

            for it in range(N_ITERS):
                x_sb = xpool.tile([128, M_BATCH, SP], F32)
                nc.sync.dma_start(
                    x_sb[:],
                    x_d[ds(it * M_TILE * M_BATCH, M_TILE * M_BATCH), :].rearrange(
                        "(c p) f -> p c f", p=128
                    ),
                )
                o_sb = opool.tile([128, M_BATCH, S], F32, tag="o")
                for sub in range(M_BATCH):
                    tpa = tpsum.tile([128, 384], F32, tag="tp")
                    tpb = tpsum.tile([128, 384], F32, tag="tp")
                    for j in range(6):
                        ksz = T_SIZES[j]
                        src = x_sb[:, sub, ds(128 * j, ksz)]
                        dst = (tpa if j < 3 else tpb)[0:ksz, ts(j % 3, 128)]
                        nc.tensor.transpose(dst, src, ident[:])

                    xta = xtpool.tile([128, 384], F32R, tag="xta")
                    xtb = xtpool.tile([128, 384], F32R, tag="xtb")
                    nc.vector.tensor_copy(xta[:], tpa[:])
                    nc.vector.tensor_copy(xtb[:, 0:256], tpb[:, 0:256])
                    nc.vector.tensor_copy(xtb[0:92, 256:384], tpb[0:92, 256:384])

                    psA = opsum.tile([128, T0], F32, tag="psA")
                    psB = opsum.tile([128, T1], F32, tag="psB")
                    for ps_t, t0, tw, plan in [
                        (psA, 0, T0, MM_A),
                        (psB, T1_OFF, T1, MM_B),
                    ]:
                        for n, (j, p0, ksz) in enumerate(plan):
                            lhsT = (xta if j < 3 else xtb)[
                                p0 : p0 + ksz, ts(j % 3, 128)
                            ]
                            rhs = attn_all[p0 : p0 + ksz, j, ds(t0, tw)]
                            nc.tensor.matmul(
                                ps_t[:],
                                lhsT,
                                rhs,
                                start=(n == 0),
                                stop=(n == len(plan) - 1),
                            )

                    nc.scalar.activation(o_sb[:, sub, 0:T0], psA[:], GELU)
                    nc.scalar.activation(o_sb[:, sub, T0:S], psB[:, 1:T1], GELU)
                nc.sync.dma_start(
                    out_d[ds(it * M_TILE * M_BATCH, M_TILE * M_BATCH), :].rearrange(
                        "(c p) f -> p c f", p=128
                    ),
                    o_sb[:],
                )

    nc.compile()
    return nc


def kernel(x, w1, w2, b2, sparse_mask):
    global _COMPILED, LAST
    if _COMPILED is None:
        _COMPILED = _build()
    nc = _COMPILED

    x = np.asarray(x, dtype=np.float32)
    w1 = np.asarray(w1, dtype=np.float32)
    w2 = np.asarray(w2, dtype=np.float32)
    b2 = np.asarray(b2, dtype=np.float32)
    mask = np.asarray(sparse_mask, dtype=np.float32)

    # host-side layout prep (no FLOPs): transpose w2 to [d, t], pad the
    # hidden dim to 8*384 for uniform per-core d-shards, slice the mask's
    # 9 diagonal 81x81 blocks, insert/append the ones columns into x
    w1p = np.zeros((HP, S), np.float32)
    w1p[:H] = w1
    w2tp = np.zeros((HP, S), np.float32)
    w2tp[:H] = np.ascontiguousarray(w2.T)
    maskb = np.stack(
        [mask[i * BLK : (i + 1) * BLK, i * BLK : (i + 1) * BLK] for i in range(NBLK)]
    )
    b2r = np.ascontiguousarray(b2.reshape(1, S))

    xf = x.reshape(B * D, S)
    xt = np.zeros((B * D, SP), np.float32)
    xt[:, 0:405] = xf[:, 0:405]
    xt[:, 405] = 1.0
    xt[:, 406:730] = xf[:, 405:729]
    xt[:, 730] = 1.0
    in_maps = []
    for c in range(N_CORES):
        in_maps.append(
            {
                "x": xt[c * M_PER_CORE : (c + 1) * M_PER_CORE],
                "w1c": np.ascontiguousarray(w1p[c * HC : (c + 1) * HC]),
                "w2tc": np.ascontiguousarray(w2tp[c * HC : (c + 1) * HC]),
                "maskb": maskb,
                "b2": b2r,
            }
        )

    LAST = run_bass_kernel_spmd(nc, in_maps, list(range(N_CORES)))
    out = np.concatenate([LAST.results[c]["out"] for c in range(N_CORES)], axis=0)
    return out.reshape(B, D, S).astype(np.float32, copy=False)


# revision 18
# speedup vs baseline: 1.0412x; 1.0412x over previous
"""Trainium2 Bass kernel for nn_ButterflyFactorNewMlp.

Computes: attn = einsum('ds,td->st', w1, w2) * sparse_mask
          out  = gelu(einsum('bds,st->bdt', x, attn) + b2)   (exact erf gelu)

Key structural facts (hardcoded):
  - x: [64, 768, 729] f32; w1: [2916, 729]; w2: [729, 2916]; b2: [729]
  - sparse_mask is block-diagonal: mask[s,t] != 0  iff  s//81 == t//81 and
    (s%27)//3 == (t%27)//3.  So attn = blockdiag of 9 diagonal 81x81 blocks.
  - Sharding: data-parallel on batch (8 batches/core); the attn computation
    is sharded over the hidden (d) dim with an on-device AllReduce of the
    9 diagonal blocks (236KB).

Contraction layout (s' space, 732 wide): the host inserts a ones column at
position 405 and appends another at 730, so after the on-chip transpose the
bias rows land at s'=405 (serving output cols 0:384) and s'=730 (cols
384:729).  Features s >= 405 shift by one.  With attn block-diagonal, the
output column range [0,384) only needs contraction rows s' < 406 and
[384,729) only rows s' >= 320, so each output half needs just 4 matmuls:

  half A (t 0:384):   s'-chunks 0,1,2 full (K=128) + chunk3 rows 0:22
  half B (t 383:729): chunk2 rows 64:128 + chunks 3,4 full + chunk5 rows 0:91

Per 128-token tile: 6 PE transposes (fp32r) put s' on partitions, 8 fp32r
matmuls contract against the block-diagonal attn (bias rows ride along via
the ones columns), exact-gelu via ScalarE LUT straight out of PSUM.
"""

import sys

if "/opt/trn_rl_repo" not in sys.path:
    sys.path.insert(0, "/opt/trn_rl_repo")

import numpy as np

import concourse.bacc as bacc
import concourse.bass as bass
import concourse.mybir as mybir
import concourse.tile as tile
from concourse.bass import ds, ts
from concourse.bass_utils import run_bass_kernel_spmd
from concourse.masks import make_identity

F32 = mybir.dt.float32
F32R = mybir.dt.float32r
GELU = mybir.ActivationFunctionType.Gelu

N_CORES = 8
B, D, S = 64, 768, 729          # batch, channels, features (729 = in = out)
H = 2916                        # hidden dim of the weight contraction
SP = 732                        # s' width: 729 + 2 ones columns + 1 pad
M_PER_CORE = (B // N_CORES) * D  # 6144 tokens per core
M_TILE = 128
M_BATCH = 2                      # m-tiles DMA'd per transfer (~750KB)
N_ITERS = M_PER_CORE // (M_TILE * M_BATCH)
HP = 3072                        # hidden padded to 8*384
HC = HP // N_CORES               # 384 hidden rows per core
N_KD = HC // 128                 # 3 contraction chunks for the attn matmuls
NBLK = 9                         # 81x81 diagonal blocks
BLK = 81
T_SIZES = [128, 128, 128, 128, 128, 92]  # transpose widths per s'-chunk
# main-matmul plan: (chunk j, partition base, K) per output half
MM_A = [(0, 0, 128), (1, 0, 128), (2, 0, 128), (3, 0, 22)]
MM_B = [(2, 64, 64), (3, 0, 128), (4, 0, 128), (5, 0, 91)]
# t split: both halves >= 256 keeps fp32r at full rate; fp32r needs an even
# psum free count and 729 is odd, so half B is 346 wide starting at 383
# (column 383 computed twice, copied out once)
T0 = 384
T1_OFF, T1 = 383, 346

_COMPILED = None
LAST = None  # BassKernelResults of the most recent kernel() call (for test.py)


def _build():
    nc = bacc.Bacc("TRN2", target_bir_lowering=False, debug=False)

    x_d = nc.dram_tensor("x", [M_PER_CORE, SP], F32, kind="ExternalInput")
    w1_d = nc.dram_tensor("w1c", [HC, S], F32, kind="ExternalInput")
    w2t_d = nc.dram_tensor("w2tc", [HC, S], F32, kind="ExternalInput")
    mb_d = nc.dram_tensor("maskb", [NBLK, BLK, BLK], F32, kind="ExternalInput")
    b2_d = nc.dram_tensor("b2", [1, S], F32, kind="ExternalInput")
    out_d = nc.dram_tensor("out", [M_PER_CORE, S], F32, kind="ExternalOutput")
    part_d = nc.dram_tensor("attn_part", [NBLK, BLK, BLK], F32)
    ar_d = nc.dram_tensor("attn_ar", [NBLK, BLK, BLK], F32, addr_space="Shared")

    with tile.TileContext(nc) as tc:
        with (
            tc.tile_pool(name="const", bufs=1) as cpool,
            tc.tile_pool(name="xin", bufs=3) as xpool,
            tc.tile_pool(name="xt", bufs=30) as xtpool,
            tc.tile_pool(name="oout", bufs=3) as opool,
            tc.tile_pool(name="tpsum", bufs=4, space="PSUM") as tpsum,
            tc.tile_pool(name="opsum", bufs=2, space="PSUM") as opsum,
        ):
            # ---------------- stage 1: attn blocks ----------------
            # fp32 matmuls here (fp32r forbids odd 81-wide outputs, and at
            # ~4us total the 4-cycle fp32 rate is irrelevant)
            w1_sb = cpool.tile([128, N_KD, S], F32)
            w2_sb = cpool.tile([128, N_KD, S], F32)
            # one DMA per 128-row chunk so the first attn matmuls can start
            # as soon as chunk 0 of both weights has landed
            for kd in range(N_KD):
                nc.sync.dma_start(w1_sb[:, kd, :], w1_d[ts(kd, 128), :])
                nc.sync.dma_start(w2_sb[:, kd, :], w2t_d[ts(kd, 128), :])
            mb_sb = cpool.tile([BLK, NBLK, BLK], F32)
            nc.sync.dma_start(mb_sb[:], mb_d[:].rearrange("b p f -> p b f"))

            part_sb = cpool.tile([BLK, NBLK, BLK], F32)
            for i in range(NBLK):
                ps_t = opsum.tile([BLK, BLK], F32, tag="psA")
                for kd in range(N_KD):
                    nc.tensor.matmul(
                        ps_t[:],
                        w1_sb[:, kd, ts(i, BLK)],
                        w2_sb[:, kd, ts(i, BLK)],
                        start=(kd == 0),
                        stop=(kd == N_KD - 1),
                    )
                nc.vector.tensor_tensor(
                    part_sb[:, i, :], ps_t[:], mb_sb[:, i, :], mybir.AluOpType.mult
                )
            nc.sync.dma_start(part_d[:].rearrange("b p f -> p b f"), part_sb[:])

            nc.gpsimd.collective_compute(
                "AllReduce",
                mybir.AluOpType.add,
                replica_groups=[list(range(N_CORES))],
                ins=[part_d[:]],
                outs=[ar_d[:]],
            )

            # dense block-diagonal attn in SBUF (f32r), in s' coordinates:
            # blocks 5-8 shift down one row; bias rows at s'=405 and s'=730
            attn_all = cpool.tile([128, 6, S], F32R)
            nc.gpsimd.memset(attn_all[:].bitcast(F32), 0.0)
            for i in range(NBLK):
                r0 = i * BLK + (1 if i >= 5 else 0)
                start = r0
                while start < r0 + BLK:
                    j = start // 128
                    end = min(r0 + BLK, (j + 1) * 128)
                    # cast f32 -> f32r during DMA (SWDGE); the rounding
                    # happens in the PE, identical to a DVE cast
                    nc.gpsimd.dma_start(
                        attn_all[start - j * 128 : end - j * 128, j, ts(i, BLK)],
                        ar_d[i, start - r0 : end - r0, :],
                    )
                    start = end
            nc.gpsimd.dma_start(attn_all[21:22, 3, 0:T0], b2_d[:, 0:T0])
            nc.gpsimd.dma_start(attn_all[90:91, 5, T0:S], b2_d[:, T0:S])

            # ---------------- stage 2: main matmul ----------------
            identf = cpool.tile([128, 128], F32)
            make_identity(nc, identf[:])
            ident = cpool.tile([128, 128], F32R)
            nc.vector.tensor_copy(ident[:], identf[:])

            for it in range(N_ITERS):
                x_sb = xpool.tile([128, M_BATCH, SP], F32R)
                # cast-DMA (f32 dram -> f32r sbuf) of two m-tiles at once
                nc.gpsimd.dma_start(
                    x_sb[:],
                    x_d[ds(it * M_TILE * M_BATCH, M_TILE * M_BATCH), :].rearrange(
                        "(c p) f -> p c f", p=128
                    ),
                )
                o_sb = opool.tile([128, M_BATCH, S], F32, tag="o")
                for sub in range(M_BATCH):
                    tpa = tpsum.tile([128, 384], F32R, tag="tp")
                    tpb = tpsum.tile([128, 384], F32R, tag="tp")
                    for j in range(6):
                        ksz = T_SIZES[j]
                        src = x_sb[:, sub, ds(128 * j, ksz)]
                        dst = (tpa if j < 3 else tpb)[0:ksz, ts(j % 3, 128)]
                        nc.tensor.transpose(dst, src, ident[:])

                    xta = xtpool.tile([128, 384], F32R, tag="xta")
                    xtb = xtpool.tile([128, 384], F32R, tag="xtb")
                    nc.vector.tensor_copy(xta[:], tpa[:])
                    nc.vector.tensor_copy(xtb[:, 0:256], tpb[:, 0:256])
                    nc.vector.tensor_copy(xtb[0:92, 256:384], tpb[0:92, 256:384])

                    psA = opsum.tile([128, T0], F32, tag="psA")
                    psB = opsum.tile([128, T1], F32, tag="psB")
                    for ps_t, t0, tw, plan in [
                        (psA, 0, T0, MM_A),
                        (psB, T1_OFF, T1, MM_B),
                    ]:
                        for n, (j, p0, ksz) in enumerate(plan):
                            lhsT = (xta if j < 3 else xtb)[
                                p0 : p0 + ksz, ts(j % 3, 128)
                            ]
                            rhs = attn_all[p0 : p0 + ksz, j, ds(t0, tw)]
                            nc.tensor.matmul(
                                ps_t[:],
                                lhsT,
                                rhs,
                                start=(n == 0),
                                stop=(n == len(plan) - 1),
                            )

                    nc.scalar.activation(o_sb[:, sub, 0:T0], psA[:], GELU)
                    nc.scalar.activation(o_sb[:, sub, T0:S], psB[:, 1:T1], GELU)
                nc.sync.dma_start(
                    out_d[ds(it * M_TILE * M_BATCH, M_TILE * M_BATCH), :].rearrange(
                        "(c p) f -> p c f", p=128
                    ),
                    o_sb[:],
                )

    nc.compile()
    return nc


def kernel(x, w1, w2, b2, sparse_mask):
    global _COMPILED, LAST
    if _COMPILED is None:
        _COMPILED = _build()
    nc = _COMPILED

    x = np.asarray(x, dtype=np.float32)
    w1 = np.asarray(w1, dtype=np.float32)
    w2 = np.asarray(w2, dtype=np.float32)
    b2 = np.asarray(b2, dtype=np.float32)
    mask = np.asarray(sparse_mask, dtype=np.float32)

    # host-side layout prep (no FLOPs): transpose w2 to [d, t], pad the
    # hidden dim to 8*384 for uniform per-core d-shards, slice the mask's
    # 9 diagonal 81x81 blocks, insert/append the ones columns into x
    w1p = np.zeros((HP, S), np.float32)
    w1p[:H] = w1
    w2tp = np.zeros((HP, S), np.float32)
    w2tp[:H] = np.ascontiguousarray(w2.T)
    maskb = np.stack(
        [mask[i * BLK : (i + 1) * BLK, i * BLK : (i + 1) * BLK] for i in range(NBLK)]
    )
    b2r = np.ascontiguousarray(b2.reshape(1, S))

    xf = x.reshape(B * D, S)
    xt = np.zeros((B * D, SP), np.float32)
    xt[:, 0:405] = xf[:, 0:405]
    xt[:, 405] = 1.0
    xt[:, 406:730] = xf[:, 405:729]
    xt[:, 730] = 1.0
    in_maps = []
    for c in range(N_CORES):
        in_maps.append(
            {
                "x": xt[c * M_PER_CORE : (c + 1) * M_PER_CORE],
                "w1c": np.ascontiguousarray(w1p[c * HC : (c + 1) * HC]),
                "w2tc": np.ascontiguousarray(w2tp[c * HC : (c + 1) * HC]),
                "maskb": maskb,
                "b2": b2r,
            }
        )

    LAST = run_bass_kernel_spmd(nc, in_maps, list(range(N_CORES)))
    out = np.concatenate([LAST.results[c]["out"] for c in range(N_CORES)], axis=0)
    return out.reshape(B, D, S).astype(np.float32, copy=False)


# revision 20
# speedup vs baseline: 1.3556x; 1.3020x over previous
"""Trainium2 Bass kernel for nn_ButterflyFactorNewMlp.

Computes: attn = einsum('ds,td->st', w1, w2) * sparse_mask
          out  = gelu(einsum('bds,st->bdt', x, attn) + b2)   (exact erf gelu)

Key structural facts (hardcoded):
  - x: [64, 768, 729] f32; w1: [2916, 729]; w2: [729, 2916]; b2: [729]
  - sparse_mask is block-diagonal: mask[s,t] != 0  iff  s//81 == t//81 and
    (s%27)//3 == (t%27)//3.  So attn = blockdiag of 9 diagonal 81x81 blocks.
  - Sharding: data-parallel on batch (8 batches/core); the attn computation
    is sharded over the hidden (d) dim with an on-device AllReduce of the
    9 diagonal blocks (236KB).

Precision: x is rounded to fp16 (rel err ~2.4e-4, on par with the PE's
fp32r mode which rounds attn to ~13 mantissa bits); the contraction
accumulates in fp32 PSUM and gelu runs on the fp32 accumulator, so the
end-to-end absmax error stays ~2e-4 relative to scale.

Contraction layout (s' space, 732 wide): the host inserts a ones column at
position 405 and appends another at 730, so after the on-chip transpose the
bias rows land at s'=405 (serving output cols 0:384) and s'=730 (cols
384:729).  Features s >= 405 shift by one.  With attn block-diagonal, the
output column range [0,384) only needs contraction rows s' < 406 and
[384,729) only rows s' >= 320, so each output half needs just 4 matmuls:

  half A (t 0:384):   s'-chunks 0,1,2 full (K=128) + chunk3 rows 0:22
  half B (t 383:729): chunk2 rows 64:128 + chunks 3,4 full + chunk5 rows 0:91

Per 128-token tile: 6 PE transposes (fp16, all into one PSUM bank) put s'
on partitions, one DVE copy moves them to SBUF, 8 fp16xfp32r matmuls
contract against the block-diagonal attn (bias rows ride along via the
ones columns), exact-gelu via ScalarE LUT straight out of PSUM.
"""

import sys

if "/opt/trn_rl_repo" not in sys.path:
    sys.path.insert(0, "/opt/trn_rl_repo")

import numpy as np

import concourse.bacc as bacc
import concourse.bass as bass
import concourse.mybir as mybir
import concourse.tile as tile
from concourse.bass import ds, ts
from concourse.bass_utils import run_bass_kernel_spmd
from concourse.masks import make_identity

F32 = mybir.dt.float32
F32R = mybir.dt.float32r
F16 = mybir.dt.float16
GELU = mybir.ActivationFunctionType.Gelu

N_CORES = 8
B, D, S = 64, 768, 729          # batch, channels, features (729 = in = out)
H = 2916                        # hidden dim of the weight contraction
SP = 732                        # s' width: 729 + 2 ones columns + 1 pad
M_PER_CORE = (B // N_CORES) * D  # 6144 tokens per core
M_TILE = 128
M_BATCH = 4                      # m-tiles DMA'd per transfer
N_ITERS = M_PER_CORE // (M_TILE * M_BATCH)
HP = 3072                        # hidden padded to 8*384
HC = HP // N_CORES               # 384 hidden rows per core
N_KD = HC // 128                 # 3 contraction chunks for the attn matmuls
NBLK = 9                         # 81x81 diagonal blocks
BLK = 81
T_SIZES = [128, 128, 128, 128, 128, 92]  # transpose widths per s'-chunk
# main-matmul plan: (chunk j, partition base, K) per output half
MM_A = [(0, 0, 128), (1, 0, 128), (2, 0, 128), (3, 0, 22)]
MM_B = [(2, 64, 64), (3, 0, 128), (4, 0, 128), (5, 0, 91)]
# t split: fp32r rhs needs an even psum free count and 729 is odd, so half B
# is 346 wide starting at 383 (column 383 computed twice, copied out once)
T0 = 384
T1_OFF, T1 = 383, 346

_COMPILED = None
LAST = None  # BassKernelResults of the most recent kernel() call (for test.py)


def _build():
    nc = bacc.Bacc("TRN2", target_bir_lowering=False, debug=False)

    x_d = nc.dram_tensor("x", [M_PER_CORE, SP], F16, kind="ExternalInput")
    w1_d = nc.dram_tensor("w1c", [HC, S], F32, kind="ExternalInput")
    w2t_d = nc.dram_tensor("w2tc", [HC, S], F32, kind="ExternalInput")
    mb_d = nc.dram_tensor("maskb", [NBLK, BLK, BLK], F32, kind="ExternalInput")
    b2_d = nc.dram_tensor("b2", [1, S], F32, kind="ExternalInput")
    out_d = nc.dram_tensor("out", [M_PER_CORE, S], F32, kind="ExternalOutput")
    part_d = nc.dram_tensor("attn_part", [NBLK, BLK, BLK], F32)
    ar_d = nc.dram_tensor("attn_ar", [NBLK, BLK, BLK], F32, addr_space="Shared")

    with tile.TileContext(nc) as tc:
        with (
            tc.tile_pool(name="const", bufs=1) as cpool,
            tc.tile_pool(name="xin", bufs=3) as xpool,
            tc.tile_pool(name="xt", bufs=40) as xtpool,
            tc.tile_pool(name="oout", bufs=3) as opool,
            tc.tile_pool(name="tpsum", bufs=4, space="PSUM") as tpsum,
            tc.tile_pool(name="opsum", bufs=2, space="PSUM") as opsum,
        ):
            # ---------------- stage 1: attn blocks ----------------
            # fp32 matmuls here (fp32r forbids odd 81-wide outputs, and at
            # ~4us total the 4-cycle fp32 rate is irrelevant)
            w1_sb = cpool.tile([128, N_KD, S], F32)
            w2_sb = cpool.tile([128, N_KD, S], F32)
            for kd in range(N_KD):
                nc.sync.dma_start(w1_sb[:, kd, :], w1_d[ts(kd, 128), :])
                nc.sync.dma_start(w2_sb[:, kd, :], w2t_d[ts(kd, 128), :])
            mb_sb = cpool.tile([BLK, NBLK, BLK], F32)
            nc.sync.dma_start(mb_sb[:], mb_d[:].rearrange("b p f -> p b f"))

            part_sb = cpool.tile([BLK, NBLK, BLK], F32)
            for i in range(NBLK):
                ps_t = opsum.tile([BLK, BLK], F32, tag="psA")
                for kd in range(N_KD):
                    nc.tensor.matmul(
                        ps_t[:],
                        w1_sb[:, kd, ts(i, BLK)],
                        w2_sb[:, kd, ts(i, BLK)],
                        start=(kd == 0),
                        stop=(kd == N_KD - 1),
                    )
                nc.vector.tensor_tensor(
                    part_sb[:, i, :], ps_t[:], mb_sb[:, i, :], mybir.AluOpType.mult
                )
            nc.sync.dma_start(part_d[:].rearrange("b p f -> p b f"), part_sb[:])

            nc.gpsimd.collective_compute(
                "AllReduce",
                mybir.AluOpType.add,
                replica_groups=[list(range(N_CORES))],
                ins=[part_d[:]],
                outs=[ar_d[:]],
            )

            # dense block-diagonal attn in SBUF (fp16 — the PE rejects mixed
            # f32r/fp16 operands and x is fp16), in s' coordinates: blocks
            # 5-8 shift down one row; bias rows at s'=405 and s'=730.
            # Chunk width padded to 730 so the f32-bitcast memset is exact.
            attn_all = cpool.tile([128, 6, S + 1], F16)
            nc.gpsimd.memset(attn_all[:].bitcast(F32), 0.0)
            for i in range(NBLK):
                r0 = i * BLK + (1 if i >= 5 else 0)
                start = r0
                while start < r0 + BLK:
                    j = start // 128
                    end = min(r0 + BLK, (j + 1) * 128)
                    # cast f32 -> f32r during DMA (SWDGE); the rounding
                    # happens in the PE, identical to a DVE cast
                    nc.gpsimd.dma_start(
                        attn_all[start - j * 128 : end - j * 128, j, ts(i, BLK)],
                        ar_d[i, start - r0 : end - r0, :],
                    )
                    start = end
            nc.gpsimd.dma_start(attn_all[21:22, 3, 0:T0], b2_d[:, 0:T0])
            nc.gpsimd.dma_start(attn_all[90:91, 5, T0:S], b2_d[:, T0:S])

            # ---------------- stage 2: main matmul ----------------
            identf = cpool.tile([128, 128], F32)
            make_identity(nc, identf[:])
            ident = cpool.tile([128, 128], F16)
            nc.vector.tensor_copy(ident[:], identf[:])

            for it in range(N_ITERS):
                x_sb = xpool.tile([128, M_BATCH, SP], F16)
                nc.sync.dma_start(
                    x_sb[:],
                    x_d[ds(it * M_TILE * M_BATCH, M_TILE * M_BATCH), :].rearrange(
                        "(c p) f -> p c f", p=128
                    ),
                )
                o_sb = opool.tile([128, M_BATCH, S], F32, tag="o")
                for sub in range(M_BATCH):
                    # all 6 transposes land in a single PSUM bank (fp16)
                    tp = tpsum.tile([128, 768], F16, tag="tp")
                    for j in range(6):
                        ksz = T_SIZES[j]
                        nc.tensor.transpose(
                            tp[0:ksz, ts(j, 128)],
                            x_sb[:, sub, ds(128 * j, ksz)],
                            ident[:],
                        )
                    xt = xtpool.tile([128, 768], F16, tag="xt")
                    nc.vector.tensor_copy(xt[:], tp[:])

                    psA = opsum.tile([128, T0], F32, tag="psA")
                    psB = opsum.tile([128, T1], F32, tag="psB")
                    for ps_t, t0, tw, plan in [
                        (psA, 0, T0, MM_A),
                        (psB, T1_OFF, T1, MM_B),
                    ]:
                        for n, (j, p0, ksz) in enumerate(plan):
                            nc.tensor.matmul(
                                ps_t[:],
                                xt[p0 : p0 + ksz, ts(j, 128)],
                                attn_all[p0 : p0 + ksz, j, ds(t0, tw)],
                                start=(n == 0),
                                stop=(n == len(plan) - 1),
                            )

                    nc.scalar.activation(o_sb[:, sub, 0:T0], psA[:], GELU)
                    nc.scalar.activation(o_sb[:, sub, T0:S], psB[:, 1:T1], GELU)
                nc.sync.dma_start(
                    out_d[ds(it * M_TILE * M_BATCH, M_TILE * M_BATCH), :].rearrange(
                        "(c p) f -> p c f", p=128
                    ),
                    o_sb[:],
                )

    nc.compile()
    return nc


def kernel(x, w1, w2, b2, sparse_mask):
    global _COMPILED, LAST
    if _COMPILED is None:
        _COMPILED = _build()
    nc = _COMPILED

    x = np.asarray(x, dtype=np.float32)
    w1 = np.asarray(w1, dtype=np.float32)
    w2 = np.asarray(w2, dtype=np.float32)
    b2 = np.asarray(b2, dtype=np.float32)
    mask = np.asarray(sparse_mask, dtype=np.float32)

    # host-side layout prep (no FLOPs): transpose w2 to [d, t], pad the
    # hidden dim to 8*384 for uniform per-core d-shards, slice the mask's
    # 9 diagonal 81x81 blocks, insert/append the ones columns into x
    w1p = np.zeros((HP, S), np.float32)
    w1p[:H] = w1
    w2tp = np.zeros((HP, S), np.float32)
    w2tp[:H] = np.ascontiguousarray(w2.T)
    maskb = np.stack(
        [mask[i * BLK : (i + 1) * BLK, i * BLK : (i + 1) * BLK] for i in range(NBLK)]
    )
    b2r = np.ascontiguousarray(b2.reshape(1, S))

    xf = x.reshape(B * D, S)
    xt = np.zeros((B * D, SP), np.float16)
    xt[:, 0:405] = xf[:, 0:405]
    xt[:, 405] = 1.0
    xt[:, 406:730] = xf[:, 405:729]
    xt[:, 730] = 1.0
    in_maps = []
    for c in range(N_CORES):
        in_maps.append(
            {
                "x": xt[c * M_PER_CORE : (c + 1) * M_PER_CORE],
                "w1c": np.ascontiguousarray(w1p[c * HC : (c + 1) * HC]),
                "w2tc": np.ascontiguousarray(w2tp[c * HC : (c + 1) * HC]),
                "maskb": maskb,
                "b2": b2r,
            }
        )

    LAST = run_bass_kernel_spmd(nc, in_maps, list(range(N_CORES)))
    out = np.concatenate([LAST.results[c]["out"] for c in range(N_CORES)], axis=0)
    return out.reshape(B, D, S).astype(np.float32, copy=False)


# revision 22
# speedup vs baseline: 1.7510x; 1.2917x over previous
"""Trainium2 Bass kernel for nn_ButterflyFactorNewMlp.

Computes: attn = einsum('ds,td->st', w1, w2) * sparse_mask
          out  = gelu(einsum('bds,st->bdt', x, attn) + b2)   (exact erf gelu)

Key structural facts (hardcoded):
  - x: [64, 768, 729] f32; w1: [2916, 729]; w2: [729, 2916]; b2: [729]
  - sparse_mask is block-diagonal: mask[s,t] != 0  iff  s//81 == t//81 and
    (s%27)//3 == (t%27)//3.  So attn = blockdiag of 9 diagonal 81x81 blocks.
  - Sharding: data-parallel on batch (8 batches per core).  The small attn
    computation is replicated on every core (fp16 weights, ~8.6MB DMA) —
    measured, this beats d-sharding + AllReduce: any on-device collective
    drags in a ~20us ncfw startup + ~45us kernel-entry barrier (launch-skew
    sync) + ~18us latency-bound AllReduce, serializing ~100us before the
    attn-dependent matmuls can start.

Precision: x and the weights are rounded to fp16 (rel err ~2.4e-4 per
element, on par with the PE's fp32r mode); contractions accumulate in fp32
PSUM and gelu runs on the fp32 accumulator via the ScalarE erf-gelu LUT
(LUT error ~2e-6), so the end-to-end absmax error stays ~5e-4 relative.

Contraction layout (s' space, 732 wide): the host inserts a ones column at
position 405 and appends another at 730, so after the on-chip transpose the
bias rows land at s'=405 (serving output cols 0:384) and s'=730 (cols
384:729).  Features s >= 405 shift by one.  attn lives in SBUF as six
128-row s'-chunks of 730 t-columns, written directly by stage 1:

  stage 1 (replicated attn): for each s'-chunk j, the only t-columns any of
  its rows touch span < 244 columns (the mask is block-diagonal and chunk
  rows cover <= 3 blocks).  So chunk j is one 23-deep accumulation of
  [128d x 128s']^T @ [128d x 244t] fp16 matmuls; a DVE multiply by a
  host-precomputed mask window writes the masked result straight into the
  chunk's rows (partition-aligned, no shuffling).  b2 is cast-DMA'd into
  the two bias rows.

  stage 2: per 128-token tile, 6 PE transposes (fp16, all into one PSUM
  bank) put s' on partitions, one DVE copy moves them to SBUF, and with
  attn block-diagonal each output half needs just 4 matmuls:
    half A (t 0:384):   s'-chunks 0,1,2 full (K=128) + chunk3 rows 0:22
    half B (t 383:729): chunk2 rows 64:128 + chunks 3,4 full + chunk5 0:91
  (fp16 needs nothing even-sized, but half B stays 346 wide from 383 so
  column 383 is computed twice and copied out once — free, and it keeps
  both halves' free dims >= 256.)  Exact-gelu runs straight out of PSUM.
"""

import sys

if "/opt/trn_rl_repo" not in sys.path:
    sys.path.insert(0, "/opt/trn_rl_repo")

import numpy as np

import concourse.bacc as bacc
import concourse.bass as bass
import concourse.mybir as mybir
import concourse.tile as tile
from concourse.bass import ds, ts
from concourse.bass_utils import run_bass_kernel_spmd
from concourse.masks import make_identity

F32 = mybir.dt.float32
F16 = mybir.dt.float16
GELU = mybir.ActivationFunctionType.Gelu

N_CORES = 8
B, D, S = 64, 768, 729          # batch, channels, features (729 = in = out)
H = 2916                        # hidden dim of the weight contraction
SP = 732                        # s' width: 729 + 2 ones columns + 1 pad
M_PER_CORE = (B // N_CORES) * D  # 6144 tokens per core
M_TILE = 128
M_BATCH = 4                      # m-tiles DMA'd per transfer
N_ITERS = M_PER_CORE // (M_TILE * M_BATCH)
HP = 2944                        # hidden padded to 23*128
N_KD = HP // 128                 # 23 contraction chunks for the attn matmuls
KD_BATCH = 4                     # kd chunks per weight DMA (~750KB)
T_SIZES = [128, 128, 128, 128, 128, 92]  # transpose widths per s'-chunk
# stage-1 window starts: chunk j's masked columns live in [WC0[j], WC0[j]+244)
WIN = 244
WC0 = [0, 81, 243, 324, 485, 485]
# main-matmul plan: (chunk j, partition base, K) per output half
MM_A = [(0, 0, 128), (1, 0, 128), (2, 0, 128), (3, 0, 22)]
MM_B = [(2, 64, 64), (3, 0, 128), (4, 0, 128), (5, 0, 91)]
T0 = 384
T1_OFF, T1 = 383, 346

_COMPILED = None
LAST = None  # BassKernelResults of the most recent kernel() call (for test.py)


def _build():
    nc = bacc.Bacc("TRN2", target_bir_lowering=False, debug=False)

    x_d = nc.dram_tensor("x", [M_PER_CORE, SP], F16, kind="ExternalInput")
    w1_d = nc.dram_tensor("w1s", [HP, SP], F16, kind="ExternalInput")
    w2t_d = nc.dram_tensor("w2ts", [HP, S], F16, kind="ExternalInput")
    mw_d = nc.dram_tensor("maskw", [6, 128, WIN], F16, kind="ExternalInput")
    b2_d = nc.dram_tensor("b2", [1, S], F32, kind="ExternalInput")
    out_d = nc.dram_tensor("out", [M_PER_CORE, S], F32, kind="ExternalOutput")

    with tile.TileContext(nc) as tc:
        with (
            tc.tile_pool(name="const", bufs=1) as cpool,
            tc.tile_pool(name="xin", bufs=3) as xpool,
            tc.tile_pool(name="xt", bufs=10) as xtpool,
            tc.tile_pool(name="oout", bufs=3) as opool,
            tc.tile_pool(name="tpsum", bufs=4, space="PSUM") as tpsum,
            tc.tile_pool(name="opsum", bufs=2, space="PSUM") as opsum,
        ):
            # ---------------- stage 1: replicated attn ----------------
            w1_sb = cpool.tile([128, N_KD, SP], F16)
            w2_sb = cpool.tile([128, N_KD, S], F16)
            for kb in range(N_KD // KD_BATCH + 1):
                k0 = kb * KD_BATCH
                kn = min(KD_BATCH, N_KD - k0)
                if kn <= 0:
                    continue
                nc.sync.dma_start(
                    w1_sb[:, ds(k0, kn), :],
                    w1_d[ds(k0 * 128, kn * 128), :].rearrange(
                        "(c p) f -> p c f", p=128
                    ),
                )
                nc.sync.dma_start(
                    w2_sb[:, ds(k0, kn), :],
                    w2t_d[ds(k0 * 128, kn * 128), :].rearrange(
                        "(c p) f -> p c f", p=128
                    ),
                )
            mw_sb = cpool.tile([128, 6, WIN], F16)
            nc.sync.dma_start(mw_sb[:], mw_d[:].rearrange("c p f -> p c f"))

            # attn in SBUF (fp16), s' coordinates; chunk width padded to 730
            # so the f32-bitcast memset is exact
            attn_all = cpool.tile([128, 6, S + 1], F16)
            nc.gpsimd.memset(attn_all[:].bitcast(F32), 0.0)
            for j in range(6):
                cw = T_SIZES[j]  # chunk width (92 for the last chunk)
                psw = opsum.tile([128, WIN], F32, tag="psA")
                for kd in range(N_KD):
                    nc.tensor.matmul(
                        psw[0:cw, :],
                        w1_sb[:, kd, ds(128 * j, cw)],
                        w2_sb[:, kd, ds(WC0[j], WIN)],
                        start=(kd == 0),
                        stop=(kd == N_KD - 1),
                    )
                nc.vector.tensor_tensor(
                    attn_all[0:cw, j, ds(WC0[j], WIN)],
                    psw[0:cw, :],
                    mw_sb[0:cw, j, :],
                    mybir.AluOpType.mult,
                )
            # bias rows (cast f32 -> f16 during DMA, SWDGE)
            nc.gpsimd.dma_start(attn_all[21:22, 3, 0:T0], b2_d[:, 0:T0])
            nc.gpsimd.dma_start(attn_all[90:91, 5, T0:S], b2_d[:, T0:S])

            # ---------------- stage 2: main matmul ----------------
            identf = cpool.tile([128, 128], F32)
            make_identity(nc, identf[:])
            ident = cpool.tile([128, 128], F16)
            nc.vector.tensor_copy(ident[:], identf[:])

            for it in range(N_ITERS):
                x_sb = xpool.tile([128, M_BATCH, SP], F16)
                nc.sync.dma_start(
                    x_sb[:],
                    x_d[ds(it * M_TILE * M_BATCH, M_TILE * M_BATCH), :].rearrange(
                        "(c p) f -> p c f", p=128
                    ),
                )
                o_sb = opool.tile([128, M_BATCH, S], F32, tag="o")
                for sub in range(M_BATCH):
                    # all 6 transposes land in a single PSUM bank (fp16)
                    tp = tpsum.tile([128, 768], F16, tag="tp")
                    for j in range(6):
                        ksz = T_SIZES[j]
                        nc.tensor.transpose(
                            tp[0:ksz, ts(j, 128)],
                            x_sb[:, sub, ds(128 * j, ksz)],
                            ident[:],
                        )
                    xt = xtpool.tile([128, 768], F16, tag="xt")
                    nc.vector.tensor_copy(xt[:], tp[:])

                    psA = opsum.tile([128, T0], F32, tag="psA")
                    psB = opsum.tile([128, T1], F32, tag="psB")
                    for ps_t, t0, tw, plan in [
                        (psA, 0, T0, MM_A),
                        (psB, T1_OFF, T1, MM_B),
                    ]:
                        for n, (j, p0, ksz) in enumerate(plan):
                            nc.tensor.matmul(
                                ps_t[:],
                                xt[p0 : p0 + ksz, ts(j, 128)],
                                attn_all[p0 : p0 + ksz, j, ds(t0, tw)],
                                start=(n == 0),
                                stop=(n == len(plan) - 1),
                            )

                    nc.scalar.activation(o_sb[:, sub, 0:T0], psA[:], GELU)
                    nc.scalar.activation(o_sb[:, sub, T0:S], psB[:, 1:T1], GELU)
                nc.sync.dma_start(
                    out_d[ds(it * M_TILE * M_BATCH, M_TILE * M_BATCH), :].rearrange(
                        "(c p) f -> p c f", p=128
                    ),
                    o_sb[:],
                )

    nc.compile()
    return nc


def _host_prep(w1, w2, mask):
    """Build the s'-layout fp16 weight/mask-window tables (layout only)."""
    w1s = np.zeros((HP, SP), np.float16)
    w1s[:H, 0:405] = w1[:, 0:405]
    w1s[:H, 406:730] = w1[:, 405:729]
    w2ts = np.zeros((HP, S), np.float16)
    w2ts[:H] = w2.T
    maskw = np.zeros((6, 128, WIN), np.float16)
    for j in range(6):
        for p in range(128):
            sp = 128 * j + p
            if sp < 405:
                s = sp
            elif 406 <= sp <= 729:
                s = sp - 1
            else:
                continue  # bias/pad rows
            maskw[j, p, :] = mask[s, WC0[j] : WC0[j] + WIN]
    return w1s, w2ts, maskw


def kernel(x, w1, w2, b2, sparse_mask):
    global _COMPILED, LAST
    if _COMPILED is None:
        _COMPILED = _build()
    nc = _COMPILED

    x = np.asarray(x, dtype=np.float32)
    w1 = np.asarray(w1, dtype=np.float32)
    w2 = np.asarray(w2, dtype=np.float32)
    b2 = np.asarray(b2, dtype=np.float32)
    mask = np.asarray(sparse_mask, dtype=np.float32)

    w1s, w2ts, maskw = _host_prep(w1, w2, mask)
    b2r = np.ascontiguousarray(b2.reshape(1, S))

    xf = x.reshape(B * D, S)
    xt = np.zeros((B * D, SP), np.float16)
    xt[:, 0:405] = xf[:, 0:405]
    xt[:, 405] = 1.0
    xt[:, 406:730] = xf[:, 405:729]
    xt[:, 730] = 1.0
    in_maps = []
    for c in range(N_CORES):
        in_maps.append(
            {
                "x": xt[c * M_PER_CORE : (c + 1) * M_PER_CORE],
                "w1s": w1s,
                "w2ts": w2ts,
                "maskw": maskw,
                "b2": b2r,
            }
        )

    LAST = run_bass_kernel_spmd(nc, in_maps, list(range(N_CORES)))
    out = np.concatenate([LAST.results[c]["out"] for c in range(N_CORES)], axis=0)
    return out.reshape(B, D, S).astype(np.float32, copy=False)
